# revision 17
# baseline (speedup 1.0000x reference)
"""AutomatonPELayer kernel for 8 Trainium2 NeuronCores.

Math: pe[j] = T^j @ x0 (j = 0..L-1), out = pe @ W.T + b, with T orthogonal
[128,128], L = 131072, embed dim 512, fp32.

Strategy (sequence-sharded, fp16 output stores):
- Row r of the output is (T^r x0)^T W^T. A PE matmul with stationary
  anchor A (A[:,p] = T^(base + 8p) x0) and moving weights
  W_r = (T^r)^T W^T produces psum[p, e] = out[base + 8p + r, e].
  Sweeping r = 0..7 with one anchor fills a 1024-row window where
  partition p holds 8 CONSECUTIVE rows (8p..8p+7) — so the SBUF->HBM
  store of a window is 128 descriptors of 8 KB contiguous DRAM each,
  the regime where the DMA engines hit peak bytes/ns.
- Outputs are stored as fp16 (the host widens to fp32 afterwards),
  halving HBM write traffic: 16384x512x2 = 16.8 MB per core. Matmul
  operands are fp16 as well (rel err ~3e-4 overall, gate is 2e-2).
- Host (float64) precompute: per-core anchors (16 per core, advancing
  by T^1024; core m offset by T^(16384 m)) and the 8 shifted weight
  matrices W_r, shipped s-major so input DMAs are 4-8 KB/partition.
- PSUM->SBUF drains (with the f32->f16 cast) are split across DVE,
  ACT and Pool so no single engine paces the pipeline; stores alternate
  between the SP and ACT HWDGE queues.
- b is folded in on the host only if nonzero (it is zero in this
  problem's setup_inputs); the device path is a pure GEMM.
"""

import sys

if "/opt/trn_rl_repo" not in sys.path:
    sys.path.insert(0, "/opt/trn_rl_repo")

import numpy as np

L = 131072
S = 128  # num states (= partition dim = contraction dim)
E = 512  # embed dim
NCORES = 8
CHUNK = L // NCORES  # 16384 rows per core
R = 8  # row interleave: rows per partition per window (8 KB f16 contiguous)
WROWS = S * R  # 1024 rows per window
WINDOWS = CHUNK // WROWS  # 16 windows per core

_prog_cache = {}


def _split_multi_waits(nc, mybir):
    """This walrus build accepts only ONE sync-wait per instruction
    (setupSyncWait: 'Too many sync wait commands'). Tile attaches the
    full wait list to the consuming instruction; hoist all but the
    last wait onto single-wait NoOps placed immediately before it on
    the same engine, preserving per-engine program order."""
    uid = 0
    for fn in nc.m.functions:
        for bb in fn.blocks:
            new = []
            changed = False
            for inst in bb.instructions:
                si = inst.sync_info
                waits = list(si.on_wait) if si is not None else []
                if len(waits) > 1:
                    changed = True
                    for w in waits[:-1]:
                        nop = mybir.InstNoOp(
                            name=f"splitw_{uid}",
                            engine=inst.engine,
                            sync_info=mybir.SyncInfo(on_wait=[w], on_update=[]),
                            bass_nofuse=True,
                        )
                        uid += 1
                        new.append(nop)
                    si.on_wait = [waits[-1]]
                new.append(inst)
            if changed:
                bb.instructions = new


def _build_program():
    if "nc" in _prog_cache:
        return _prog_cache["nc"]

    import concourse.bass as bass
    import concourse.tile as tile
    from concourse import mybir

    f32 = mybir.dt.float32
    f16 = mybir.dt.float16
    nc = bass.Bass("TRN2", target_bir_lowering=False, debug=False, num_devices=NCORES)

    # s-major layouts so each input DMA moves 4-8 KB contiguous per
    # partition. anchors differ per core; wgs replicated.
    anchors = nc.dram_tensor("anchors", [S, WINDOWS, S], f16, kind="ExternalInput").ap()
    wgs = nc.dram_tensor("wgs", [S, R, E], f16, kind="ExternalInput").ap()
    out = nc.dram_tensor("out", [CHUNK, E], f16, kind="ExternalOutput").ap()
    # window w, partition p holds rows 1024w + 8p .. 8p+7 -> 8 KB contiguous
    out_v = out.rearrange("(w p r) e -> w p (r e)", p=S, r=R)

    with tile.TileContext(nc) as tc:
        with (
            tc.tile_pool(name="singles", bufs=1) as singles,
            tc.tile_pool(name="opool", bufs=5) as opool,
            tc.tile_pool(name="psum", bufs=2, space="PSUM") as psum,
        ):
            wg_t = singles.tile([S, R, E], f16)
            anch_t = singles.tile([S, WINDOWS, S], f16)
            # Head loads: window 0's anchor + the weight PAIRS in matmul
            # order on the two fast HWDGE queues (the first matmul pair only
            # needs W_0/W_1 — don't gate it on the full 1 MB weight load);
            # remaining anchors trail on the gpsimd SWDGE queue.
            nc.scalar.dma_start(out=anch_t[:, 0, :], in_=anchors[:, 0, :])
            nc.sync.dma_start(out=wg_t, in_=wgs)
            nc.gpsimd.dma_start(out=anch_t[:, 1:, :], in_=anchors[:, 1:, :])

            # Per-window: 8 matmuls (one per row shift r) into 4 psum bank
            # PAIRS; each pair drains (with the f32->f16 cast) in one copy
            # instruction. Only DVE and ACT can read PSUM on TRN2 — split
            # pairs evenly, alternating the leadoff engine per window so
            # the ACT store dispatches stay balanced. One 1 MB store/window.
            for w in range(WINDOWS):
                o_t = opool.tile([S, R, E], f16)
                for q in range(R // 4):
                    pe4 = psum.tile([S, 4, E], f32)
                    for h in range(4):
                        nc.tensor.matmul(
                            pe4[:, h, :],
                            anch_t[:, w, :],
                            wg_t[:, 4 * q + h, :],
                            start=True,
                            stop=True,
                        )
                    # One 4-bank drain per half-window: amortizes the
                    # per-instruction overhead; DVE and ACT each take one
                    # quad per window and run concurrently.
                    if (w + q) % 2 == 0:
                        nc.vector.tensor_copy(o_t[:, 4 * q : 4 * q + 4, :], pe4)
                    else:
                        nc.scalar.copy(out=o_t[:, 4 * q : 4 * q + 4, :], in_=pe4)
                # All stores ride SP's HWDGE queue: SP is otherwise idle,
                # the queue sustains ~400 B/ns with 8 KB descriptors, and
                # store dispatches on ACT/gpsimd measurably slow the
                # PSUM-drain engines (in-order stall / SWDGE contention).
                nc.sync.dma_start(out=out_v[w], in_=o_t)

    _split_multi_waits(nc, mybir)
    _prog_cache["nc"] = nc
    return nc


def _host_precompute(pos_initial, pos_transition, W):
    """float64 host prep: stride-8 anchor blocks + shifted weights."""
    T = np.asarray(pos_transition, np.float64)
    x0 = np.asarray(pos_initial, np.float64).reshape(S)
    W64 = np.asarray(W, np.float64)

    # T^8 and T^1024 by repeated squaring
    T2 = T @ T
    T4 = T2 @ T2
    T8 = T4 @ T4
    T1024 = T8
    for _ in range(7):
        T1024 = T1024 @ T1024

    # X8[:, p] = T^(8p) x0 for p = 0..127 (stride-8 anchor base)
    X8 = np.empty((S, S), np.float64)
    v = x0.copy()
    X8[:, 0] = v
    for p in range(1, S):
        v = T8 @ v
        X8[:, p] = v

    # W_r = (T^r)^T @ W.T for r = 0..7 -> wgs[s, r, e] (s-major for DMA)
    wgs = np.empty((S, R, E), np.float64)
    Tp = np.eye(S)
    for r in range(R):
        wgs[:, r, :] = Tp.T @ W64.T
        Tp = Tp @ T
    wgs = np.ascontiguousarray(wgs).astype(np.float16)

    # anchors[m][:, w, :] = T^1024^(16m + w) @ X8, s-major
    anchors = []
    A = X8
    for m in range(NCORES):
        am = np.empty((S, WINDOWS, S), np.float64)
        for w in range(WINDOWS):
            am[:, w, :] = A
            A = T1024 @ A
        anchors.append(np.ascontiguousarray(am).astype(np.float16))
    return anchors, wgs


def kernel(sentence_len, pos_initial, pos_transition, W, b):
    from concourse.bass_utils import run_bass_kernel_spmd

    assert int(sentence_len) == L, f"kernel hardcodes L={L}, got {sentence_len}"
    b = np.asarray(b, np.float32)

    anchors, wgs = _host_precompute(pos_initial, pos_transition, W)

    nc = _build_program()
    in_maps = [{"anchors": anchors[m], "wgs": wgs} for m in range(NCORES)]
    res = run_bass_kernel_spmd(nc, in_maps, core_ids=list(range(NCORES)))
    full = np.concatenate(
        [res.results[m]["out"] for m in range(NCORES)], axis=0
    ).astype(np.float32)
    if np.any(b != 0):
        full = full + b[None, :]
    return full


# revision 21
# speedup vs baseline: 1.0499x; 1.0499x over previous
"""AutomatonPELayer kernel for 8 Trainium2 NeuronCores.

Math: pe[j] = T^j @ x0 (j = 0..L-1), out = pe @ W.T + b, with T orthogonal
[128,128], L = 131072, embed dim 512, fp32.

Strategy (sequence-sharded, fp16 output stores):
- Row r of the output is (T^r x0)^T W^T. A PE matmul with stationary
  anchor A (A[:,p] = T^(base + 8p) x0) and moving weights
  W_r = (T^r)^T W^T produces psum[p, e] = out[base + 8p + r, e].
  Sweeping r = 0..7 with one anchor fills a 1024-row window where
  partition p holds 8 CONSECUTIVE rows (8p..8p+7) — so the SBUF->HBM
  store of a window is 128 descriptors of 8 KB contiguous DRAM each,
  the regime where the DMA engines hit peak bytes/ns.
- Outputs are stored as fp16 (the host widens to fp32 afterwards),
  halving HBM write traffic: 16384x512x2 = 16.8 MB per core. Matmul
  operands are fp16 as well (rel err ~3e-4 overall, gate is 2e-2).
- Host (float64) precompute: per-core anchors (16 per core, advancing
  by T^1024; core m offset by T^(16384 m)) and the 8 shifted weight
  matrices W_r, shipped s-major so input DMAs are 4-8 KB/partition.
- PSUM->SBUF drains (with the f32->f16 cast) are split across DVE,
  ACT and Pool so no single engine paces the pipeline; stores alternate
  between the SP and ACT HWDGE queues.
- b is folded in on the host only if nonzero (it is zero in this
  problem's setup_inputs); the device path is a pure GEMM.
"""

import sys

if "/opt/trn_rl_repo" not in sys.path:
    sys.path.insert(0, "/opt/trn_rl_repo")

import numpy as np

L = 131072
S = 128  # num states (= partition dim = contraction dim)
E = 512  # embed dim
NCORES = 8
CHUNK = L // NCORES  # 16384 rows per core
R = 8  # row interleave: rows per partition per window (8 KB f16 contiguous)
WROWS = S * R  # 1024 rows per window
WINDOWS = CHUNK // WROWS  # 16 windows per core

_prog_cache = {}


def _split_multi_waits(nc, mybir):
    """This walrus build accepts only ONE sync-wait per instruction
    (setupSyncWait: 'Too many sync wait commands'). Tile attaches the
    full wait list to the consuming instruction; hoist all but the
    last wait onto single-wait NoOps placed immediately before it on
    the same engine, preserving per-engine program order."""
    uid = 0
    for fn in nc.m.functions:
        for bb in fn.blocks:
            new = []
            changed = False
            for inst in bb.instructions:
                si = inst.sync_info
                waits = list(si.on_wait) if si is not None else []
                if len(waits) > 1:
                    changed = True
                    for w in waits[:-1]:
                        nop = mybir.InstNoOp(
                            name=f"splitw_{uid}",
                            engine=inst.engine,
                            sync_info=mybir.SyncInfo(on_wait=[w], on_update=[]),
                            bass_nofuse=True,
                        )
                        uid += 1
                        new.append(nop)
                    si.on_wait = [waits[-1]]
                new.append(inst)
            if changed:
                bb.instructions = new


def _build_program():
    if "nc" in _prog_cache:
        return _prog_cache["nc"]

    import concourse.bass as bass
    import concourse.tile as tile
    from concourse import mybir

    f32 = mybir.dt.float32
    f16 = mybir.dt.float16
    nc = bass.Bass("TRN2", target_bir_lowering=False, debug=False, num_devices=NCORES)

    # s-major layouts so each input DMA moves 4-8 KB contiguous per
    # partition. anchors differ per core; wgs replicated.
    anchors = nc.dram_tensor("anchors", [S, WINDOWS, S], f16, kind="ExternalInput").ap()
    wgs = nc.dram_tensor("wgs", [S, R, E], f16, kind="ExternalInput").ap()
    out = nc.dram_tensor("out", [CHUNK, E], f16, kind="ExternalOutput").ap()
    # window w, partition p holds rows 1024w + 8p .. 8p+7 -> 8 KB contiguous
    out_v = out.rearrange("(w p r) e -> w p (r e)", p=S, r=R)

    with tile.TileContext(nc) as tc:
        with (
            tc.tile_pool(name="singles", bufs=1) as singles,
            tc.tile_pool(name="opool", bufs=5) as opool,
            tc.tile_pool(name="psum", bufs=4, space="PSUM") as psum,
        ):
            # The weight pairs live in SEPARATE tiles: Tile tracks DMA
            # writes at tile granularity, so a single wgs tile would make
            # the FIRST matmul wait for the whole 1 MB weight load. With
            # one tile per pair (loaded in matmul order on the sync queue)
            # pair 0 starts as soon as W_0/W_1 and the anchor land.
            wg_p = [
                singles.tile([S, 2, E], f16, name=f"wg_p{q}") for q in range(R // 2)
            ]
            anch_t = singles.tile([S, WINDOWS, S], f16)
            nc.scalar.dma_start(out=anch_t[:, 0, :], in_=anchors[:, 0, :])
            for q in range(R // 2):
                nc.sync.dma_start(out=wg_p[q], in_=wgs[:, 2 * q : 2 * q + 2, :])
            nc.gpsimd.dma_start(out=anch_t[:, 1:, :], in_=anchors[:, 1:, :])

            # Per-window: 8 matmuls (one per row shift r) into 4 psum bank
            # PAIRS; each pair drains (with the f32->f16 cast) in one copy
            # instruction. Only DVE and ACT can read PSUM on TRN2 — split
            # pairs evenly, alternating the leadoff engine per window so
            # the ACT store dispatches stay balanced. One 1 MB store/window.
            # Half-window view for window 0's head stores (4 KB/partition
            # segments — still full DMA efficiency).
            out_h = out.rearrange("(w p r) e -> w p r e", p=S, r=R)
            for w in range(WINDOWS):
                o_t = opool.tile([S, R, E], f16)
                for q in range(R // 2):
                    pe2 = psum.tile([S, 2, E], f32)
                    for h in range(2):
                        nc.tensor.matmul(
                            pe2[:, h, :],
                            anch_t[:, w, :],
                            wg_p[q][:, h, :],
                            start=True,
                            stop=True,
                        )
                    if (w + q) % 2 == 0:
                        nc.vector.tensor_copy(o_t[:, 2 * q : 2 * q + 2, :], pe2)
                    else:
                        nc.scalar.copy(out=o_t[:, 2 * q : 2 * q + 2, :], in_=pe2)
                    # Window 0 streams out in two half-window stores so the
                    # store queue (the longest stream) starts ~3 us earlier.
                    if w == 0 and q in (1, 3):
                        nc.sync.dma_start(
                            out=out_h[0, :, 2 * q - 2 : 2 * q + 2, :],
                            in_=o_t[:, 2 * q - 2 : 2 * q + 2, :],
                        )
                # All stores ride SP's HWDGE queue: SP is otherwise idle,
                # the queue sustains ~400 B/ns with 8 KB descriptors, and
                # store dispatches on ACT/gpsimd measurably slow the
                # PSUM-drain engines (in-order stall / SWDGE contention).
                if w > 0:
                    nc.sync.dma_start(out=out_v[w], in_=o_t)

    _split_multi_waits(nc, mybir)
    _prog_cache["nc"] = nc
    return nc


def _host_precompute(pos_initial, pos_transition, W):
    """float64 host prep: stride-8 anchor blocks + shifted weights."""
    T = np.asarray(pos_transition, np.float64)
    x0 = np.asarray(pos_initial, np.float64).reshape(S)
    W64 = np.asarray(W, np.float64)

    # T^8 and T^1024 by repeated squaring
    T2 = T @ T
    T4 = T2 @ T2
    T8 = T4 @ T4
    T1024 = T8
    for _ in range(7):
        T1024 = T1024 @ T1024

    # X8[:, p] = T^(8p) x0 for p = 0..127 (stride-8 anchor base)
    X8 = np.empty((S, S), np.float64)
    v = x0.copy()
    X8[:, 0] = v
    for p in range(1, S):
        v = T8 @ v
        X8[:, p] = v

    # W_r = (T^r)^T @ W.T for r = 0..7 -> wgs[s, r, e] (s-major for DMA)
    wgs = np.empty((S, R, E), np.float64)
    Tp = np.eye(S)
    for r in range(R):
        wgs[:, r, :] = Tp.T @ W64.T
        Tp = Tp @ T
    wgs = np.ascontiguousarray(wgs).astype(np.float16)

    # anchors[m][:, w, :] = T^1024^(16m + w) @ X8, s-major
    anchors = []
    A = X8
    for m in range(NCORES):
        am = np.empty((S, WINDOWS, S), np.float64)
        for w in range(WINDOWS):
            am[:, w, :] = A
            A = T1024 @ A
        anchors.append(np.ascontiguousarray(am).astype(np.float16))
    return anchors, wgs


def kernel(sentence_len, pos_initial, pos_transition, W, b):
    from concourse.bass_utils import run_bass_kernel_spmd

    assert int(sentence_len) == L, f"kernel hardcodes L={L}, got {sentence_len}"
    b = np.asarray(b, np.float32)

    anchors, wgs = _host_precompute(pos_initial, pos_transition, W)

    nc = _build_program()
    in_maps = [{"anchors": anchors[m], "wgs": wgs} for m in range(NCORES)]
    res = run_bass_kernel_spmd(nc, in_maps, core_ids=list(range(NCORES)))
    full = np.concatenate(
        [res.results[m]["out"] for m in range(NCORES)], axis=0
    ).astype(np.float32)
    if np.any(b != 0):
        full = full + b[None, :]
    return full


# revision 22
# speedup vs baseline: 1.1419x; 1.0876x over previous
"""AutomatonPELayer kernel for 8 Trainium2 NeuronCores.

Math: pe[j] = T^j @ x0 (j = 0..L-1), out = pe @ W.T + b, with T orthogonal
[128,128], L = 131072, embed dim 512, fp32.

Strategy (sequence-sharded, fp16 output stores):
- Row r of the output is (T^r x0)^T W^T. A PE matmul with stationary
  anchor A (A[:,p] = T^(base + 8p) x0) and moving weights
  W_r = (T^r)^T W^T produces psum[p, e] = out[base + 8p + r, e].
  Sweeping r = 0..7 with one anchor fills a 1024-row window where
  partition p holds 8 CONSECUTIVE rows (8p..8p+7) — so the SBUF->HBM
  store of a window is 128 descriptors of 8 KB contiguous DRAM each,
  the regime where the DMA engines hit peak bytes/ns.
- Outputs are stored as fp16 (the host widens to fp32 afterwards),
  halving HBM write traffic: 16384x512x2 = 16.8 MB per core. Matmul
  operands are fp16 as well (rel err ~3e-4 overall, gate is 2e-2).
- Host (float64) precompute: per-core anchors (16 per core, advancing
  by T^1024; core m offset by T^(16384 m)) and the 8 shifted weight
  matrices W_r, shipped s-major so input DMAs are 4-8 KB/partition.
- PSUM->SBUF drains (with the f32->f16 cast) are split across DVE,
  ACT and Pool so no single engine paces the pipeline; stores alternate
  between the SP and ACT HWDGE queues.
- b is folded in on the host only if nonzero (it is zero in this
  problem's setup_inputs); the device path is a pure GEMM.
"""

import sys

if "/opt/trn_rl_repo" not in sys.path:
    sys.path.insert(0, "/opt/trn_rl_repo")

import numpy as np

L = 131072
S = 128  # num states (= partition dim = contraction dim)
E = 512  # embed dim
NCORES = 8
CHUNK = L // NCORES  # 16384 rows per core
R = 8  # row interleave: rows per partition per window (8 KB f16 contiguous)
WROWS = S * R  # 1024 rows per window
WINDOWS = CHUNK // WROWS  # 16 windows per core

_prog_cache = {}


def _split_multi_waits(nc, mybir):
    """This walrus build accepts only ONE sync-wait per instruction
    (setupSyncWait: 'Too many sync wait commands'). Tile attaches the
    full wait list to the consuming instruction; hoist all but the
    last wait onto single-wait NoOps placed immediately before it on
    the same engine, preserving per-engine program order."""
    uid = 0
    for fn in nc.m.functions:
        for bb in fn.blocks:
            new = []
            changed = False
            for inst in bb.instructions:
                si = inst.sync_info
                waits = list(si.on_wait) if si is not None else []
                if len(waits) > 1:
                    changed = True
                    for w in waits[:-1]:
                        nop = mybir.InstNoOp(
                            name=f"splitw_{uid}",
                            engine=inst.engine,
                            sync_info=mybir.SyncInfo(on_wait=[w], on_update=[]),
                            bass_nofuse=True,
                        )
                        uid += 1
                        new.append(nop)
                    si.on_wait = [waits[-1]]
                new.append(inst)
            if changed:
                bb.instructions = new


def _build_program():
    if "nc" in _prog_cache:
        return _prog_cache["nc"]

    import concourse.bass as bass
    import concourse.tile as tile
    from concourse import mybir

    f32 = mybir.dt.float32
    f16 = mybir.dt.float16
    nc = bass.Bass("TRN2", target_bir_lowering=False, debug=False, num_devices=NCORES)

    # s-major layouts so each input DMA moves 4-8 KB contiguous per
    # partition. anchors differ per core; wgs replicated.
    anchors = nc.dram_tensor("anchors", [S, WINDOWS, S], f16, kind="ExternalInput").ap()
    wgs = nc.dram_tensor("wgs", [S, R, E], f16, kind="ExternalInput").ap()
    out = nc.dram_tensor("out", [CHUNK, E], f16, kind="ExternalOutput").ap()
    # window w, partition p holds rows 1024w + 8p .. 8p+7 -> 8 KB contiguous
    out_v = out.rearrange("(w p r) e -> w p (r e)", p=S, r=R)

    with tile.TileContext(nc) as tc:
        with (
            tc.tile_pool(name="singles", bufs=1) as singles,
            tc.tile_pool(name="opool", bufs=5) as opool,
            tc.tile_pool(name="psum", bufs=4, space="PSUM") as psum,
        ):
            wg_t = singles.tile([S, R, E], f16)
            anch_t = singles.tile([S, WINDOWS, S], f16)
            # Head loads: window 0's anchor + the weight PAIRS in matmul
            # order on the two fast HWDGE queues (the first matmul pair only
            # needs W_0/W_1 — don't gate it on the full 1 MB weight load);
            # remaining anchors trail on the gpsimd SWDGE queue.
            nc.scalar.dma_start(out=anch_t[:, 0, :], in_=anchors[:, 0, :])
            nc.sync.dma_start(out=wg_t, in_=wgs)
            nc.gpsimd.dma_start(out=anch_t[:, 1:, :], in_=anchors[:, 1:, :])

            # Per-window: 8 matmuls (one per row shift r) into 4 psum bank
            # PAIRS; each pair drains (with the f32->f16 cast) in one copy
            # instruction. Only DVE and ACT can read PSUM on TRN2 — split
            # pairs evenly, alternating the leadoff engine per window so
            # the ACT store dispatches stay balanced. One 1 MB store/window.
            # Most stores ride SP's HWDGE queue (otherwise-idle engine,
            # ~400 B/ns with 8 KB descriptors). The SP queue alone lags the
            # copy pace by ~4 us, so four stores divert to ACT's queue; each
            # ACT dispatch is emitted TWO WINDOWS LATE in program order so
            # its semaphore wait is already satisfied when ACT reaches it
            # (an inline dispatch stalls ACT's in-order stream on DVE's
            # copies, which measurably serializes the pipeline).
            ACT_STORE = {4, 7, 10, 13}
            o_tiles = {}
            for w in range(WINDOWS):
                o_t = opool.tile([S, R, E], f16)
                o_tiles[w] = o_t
                for q in range(R // 2):
                    pe2 = psum.tile([S, 2, E], f32)
                    for h in range(2):
                        nc.tensor.matmul(
                            pe2[:, h, :],
                            anch_t[:, w, :],
                            wg_t[:, 2 * q + h, :],
                            start=True,
                            stop=True,
                        )
                    if (w + q) % 2 == 0:
                        nc.vector.tensor_copy(o_t[:, 2 * q : 2 * q + 2, :], pe2)
                    else:
                        nc.scalar.copy(out=o_t[:, 2 * q : 2 * q + 2, :], in_=pe2)
                if w not in ACT_STORE:
                    nc.sync.dma_start(out=out_v[w], in_=o_tiles[w])
                if w - 2 in ACT_STORE:
                    nc.scalar.dma_start(out=out_v[w - 2], in_=o_tiles[w - 2])

    _split_multi_waits(nc, mybir)
    _prog_cache["nc"] = nc
    return nc


def _host_precompute(pos_initial, pos_transition, W):
    """float64 host prep: stride-8 anchor blocks + shifted weights."""
    T = np.asarray(pos_transition, np.float64)
    x0 = np.asarray(pos_initial, np.float64).reshape(S)
    W64 = np.asarray(W, np.float64)

    # T^8 and T^1024 by repeated squaring
    T2 = T @ T
    T4 = T2 @ T2
    T8 = T4 @ T4
    T1024 = T8
    for _ in range(7):
        T1024 = T1024 @ T1024

    # X8[:, p] = T^(8p) x0 for p = 0..127 (stride-8 anchor base)
    X8 = np.empty((S, S), np.float64)
    v = x0.copy()
    X8[:, 0] = v
    for p in range(1, S):
        v = T8 @ v
        X8[:, p] = v

    # W_r = (T^r)^T @ W.T for r = 0..7 -> wgs[s, r, e] (s-major for DMA)
    wgs = np.empty((S, R, E), np.float64)
    Tp = np.eye(S)
    for r in range(R):
        wgs[:, r, :] = Tp.T @ W64.T
        Tp = Tp @ T
    wgs = np.ascontiguousarray(wgs).astype(np.float16)

    # anchors[m][:, w, :] = T^1024^(16m + w) @ X8, s-major
    anchors = []
    A = X8
    for m in range(NCORES):
        am = np.empty((S, WINDOWS, S), np.float64)
        for w in range(WINDOWS):
            am[:, w, :] = A
            A = T1024 @ A
        anchors.append(np.ascontiguousarray(am).astype(np.float16))
    return anchors, wgs


def kernel(sentence_len, pos_initial, pos_transition, W, b):
    from concourse.bass_utils import run_bass_kernel_spmd

    assert int(sentence_len) == L, f"kernel hardcodes L={L}, got {sentence_len}"
    b = np.asarray(b, np.float32)

    anchors, wgs = _host_precompute(pos_initial, pos_transition, W)

    nc = _build_program()
    in_maps = [{"anchors": anchors[m], "wgs": wgs} for m in range(NCORES)]
    res = run_bass_kernel_spmd(nc, in_maps, core_ids=list(range(NCORES)))
    full = np.concatenate(
        [res.results[m]["out"] for m in range(NCORES)], axis=0
    ).astype(np.float32)
    if np.any(b != 0):
        full = full + b[None, :]
    return full


# revision 23
# speedup vs baseline: 1.1966x; 1.0480x over previous
"""AutomatonPELayer kernel for 8 Trainium2 NeuronCores.

Math: pe[j] = T^j @ x0 (j = 0..L-1), out = pe @ W.T + b, with T orthogonal
[128,128], L = 131072, embed dim 512, fp32.

Strategy (sequence-sharded, fp16 output stores):
- Row r of the output is (T^r x0)^T W^T. A PE matmul with stationary
  anchor A (A[:,p] = T^(base + 8p) x0) and moving weights
  W_r = (T^r)^T W^T produces psum[p, e] = out[base + 8p + r, e].
  Sweeping r = 0..7 with one anchor fills a 1024-row window where
  partition p holds 8 CONSECUTIVE rows (8p..8p+7) — so the SBUF->HBM
  store of a window is 128 descriptors of 8 KB contiguous DRAM each,
  the regime where the DMA engines hit peak bytes/ns.
- Outputs are stored as fp16 (the host widens to fp32 afterwards),
  halving HBM write traffic: 16384x512x2 = 16.8 MB per core. Matmul
  operands are fp16 as well (rel err ~3e-4 overall, gate is 2e-2).
- Host (float64) precompute: per-core anchors (16 per core, advancing
  by T^1024; core m offset by T^(16384 m)) and the 8 shifted weight
  matrices W_r, shipped s-major so input DMAs are 4-8 KB/partition.
- PSUM->SBUF drains (with the f32->f16 cast) are split across DVE,
  ACT and Pool so no single engine paces the pipeline; stores alternate
  between the SP and ACT HWDGE queues.
- b is folded in on the host only if nonzero (it is zero in this
  problem's setup_inputs); the device path is a pure GEMM.
"""

import sys

if "/opt/trn_rl_repo" not in sys.path:
    sys.path.insert(0, "/opt/trn_rl_repo")

import numpy as np

L = 131072
S = 128  # num states (= partition dim = contraction dim)
E = 512  # embed dim
NCORES = 8
CHUNK = L // NCORES  # 16384 rows per core
R = 8  # row interleave: rows per partition per window (8 KB f16 contiguous)
WROWS = S * R  # 1024 rows per window
WINDOWS = CHUNK // WROWS  # 16 windows per core

_prog_cache = {}


def _split_multi_waits(nc, mybir):
    """This walrus build accepts only ONE sync-wait per instruction
    (setupSyncWait: 'Too many sync wait commands'). Tile attaches the
    full wait list to the consuming instruction; hoist all but the
    last wait onto single-wait NoOps placed immediately before it on
    the same engine, preserving per-engine program order."""
    uid = 0
    for fn in nc.m.functions:
        for bb in fn.blocks:
            new = []
            changed = False
            for inst in bb.instructions:
                si = inst.sync_info
                waits = list(si.on_wait) if si is not None else []
                if len(waits) > 1:
                    changed = True
                    for w in waits[:-1]:
                        nop = mybir.InstNoOp(
                            name=f"splitw_{uid}",
                            engine=inst.engine,
                            sync_info=mybir.SyncInfo(on_wait=[w], on_update=[]),
                            bass_nofuse=True,
                        )
                        uid += 1
                        new.append(nop)
                    si.on_wait = [waits[-1]]
                new.append(inst)
            if changed:
                bb.instructions = new


def _build_program():
    if "nc" in _prog_cache:
        return _prog_cache["nc"]

    import concourse.bass as bass
    import concourse.tile as tile
    from concourse import mybir

    f32 = mybir.dt.float32
    f16 = mybir.dt.float16
    nc = bass.Bass("TRN2", target_bir_lowering=False, debug=False, num_devices=NCORES)

    # s-major layouts so each input DMA moves 4-8 KB contiguous per
    # partition. anchors differ per core; wgs replicated.
    anchors = nc.dram_tensor("anchors", [S, WINDOWS, S], f16, kind="ExternalInput").ap()
    wgs = nc.dram_tensor("wgs", [S, R, E], f16, kind="ExternalInput").ap()
    out = nc.dram_tensor("out", [CHUNK, E], f16, kind="ExternalOutput").ap()
    # window w, partition p holds rows 1024w + 8p .. 8p+7 -> 8 KB contiguous
    out_v = out.rearrange("(w p r) e -> w p (r e)", p=S, r=R)

    with tile.TileContext(nc) as tc:
        with (
            tc.tile_pool(name="singles", bufs=1) as singles,
            tc.tile_pool(name="opool", bufs=5) as opool,
            tc.tile_pool(name="psum", bufs=4, space="PSUM") as psum,
        ):
            wg_t = singles.tile([S, R, E], f16)
            anch_t = singles.tile([S, WINDOWS, S], f16)
            # Head loads: window 0's anchor + the weight PAIRS in matmul
            # order on the two fast HWDGE queues (the first matmul pair only
            # needs W_0/W_1 — don't gate it on the full 1 MB weight load);
            # remaining anchors trail on the gpsimd SWDGE queue.
            nc.scalar.dma_start(out=anch_t[:, 0, :], in_=anchors[:, 0, :])
            nc.sync.dma_start(out=wg_t, in_=wgs)
            nc.gpsimd.dma_start(out=anch_t[:, 1:, :], in_=anchors[:, 1:, :])

            # Per-window: 8 matmuls (one per row shift r) into 4 psum bank
            # PAIRS; each pair drains (with the f32->f16 cast) in one copy
            # instruction. Only DVE and ACT can read PSUM on TRN2 — split
            # pairs evenly, alternating the leadoff engine per window so
            # the ACT store dispatches stay balanced. One 1 MB store/window.
            out_h = out.rearrange("(w p r) e -> w p r e", p=S, r=R)
            for w in range(WINDOWS):
                o_t = opool.tile([S, R, E], f16)
                for q in range(R // 2):
                    pe2 = psum.tile([S, 2, E], f32)
                    for h in range(2):
                        nc.tensor.matmul(
                            pe2[:, h, :],
                            anch_t[:, w, :],
                            wg_t[:, 2 * q + h, :],
                            start=True,
                            stop=True,
                        )
                    if (w + q) % 2 == 0:
                        nc.vector.tensor_copy(o_t[:, 2 * q : 2 * q + 2, :], pe2)
                    else:
                        nc.scalar.copy(out=o_t[:, 2 * q : 2 * q + 2, :], in_=pe2)
                # All stores ride SP's HWDGE queue: SP is otherwise idle,
                # the queue sustains ~400 B/ns with 8 KB descriptors, and
                # store dispatches on ACT/gpsimd measurably slow the
                # PSUM-drain engines (in-order stall / SWDGE contention).
                # Window 0 goes out in two 4 KB-segment halves so the store
                # stream (the longest-running resource) starts earlier.
                if w == 0:
                    nc.sync.dma_start(out=out_h[0, :, 0:4, :], in_=o_t[:, 0:4, :])
                    nc.sync.dma_start(out=out_h[0, :, 4:8, :], in_=o_t[:, 4:8, :])
                else:
                    nc.sync.dma_start(out=out_v[w], in_=o_t)

    _split_multi_waits(nc, mybir)
    _prog_cache["nc"] = nc
    return nc


def _host_precompute(pos_initial, pos_transition, W):
    """float64 host prep: stride-8 anchor blocks + shifted weights."""
    T = np.asarray(pos_transition, np.float64)
    x0 = np.asarray(pos_initial, np.float64).reshape(S)
    W64 = np.asarray(W, np.float64)

    # T^8 and T^1024 by repeated squaring
    T2 = T @ T
    T4 = T2 @ T2
    T8 = T4 @ T4
    T1024 = T8
    for _ in range(7):
        T1024 = T1024 @ T1024

    # X8[:, p] = T^(8p) x0 for p = 0..127 (stride-8 anchor base)
    X8 = np.empty((S, S), np.float64)
    v = x0.copy()
    X8[:, 0] = v
    for p in range(1, S):
        v = T8 @ v
        X8[:, p] = v

    # W_r = (T^r)^T @ W.T for r = 0..7 -> wgs[s, r, e] (s-major for DMA)
    wgs = np.empty((S, R, E), np.float64)
    Tp = np.eye(S)
    for r in range(R):
        wgs[:, r, :] = Tp.T @ W64.T
        Tp = Tp @ T
    wgs = np.ascontiguousarray(wgs).astype(np.float16)

    # anchors[m][:, w, :] = T^1024^(16m + w) @ X8, s-major
    anchors = []
    A = X8
    for m in range(NCORES):
        am = np.empty((S, WINDOWS, S), np.float64)
        for w in range(WINDOWS):
            am[:, w, :] = A
            A = T1024 @ A
        anchors.append(np.ascontiguousarray(am).astype(np.float16))
    return anchors, wgs


def kernel(sentence_len, pos_initial, pos_transition, W, b):
    from concourse.bass_utils import run_bass_kernel_spmd

    assert int(sentence_len) == L, f"kernel hardcodes L={L}, got {sentence_len}"
    b = np.asarray(b, np.float32)

    anchors, wgs = _host_precompute(pos_initial, pos_transition, W)

    nc = _build_program()
    in_maps = [{"anchors": anchors[m], "wgs": wgs} for m in range(NCORES)]
    res = run_bass_kernel_spmd(nc, in_maps, core_ids=list(range(NCORES)))
    full = np.concatenate(
        [res.results[m]["out"] for m in range(NCORES)], axis=0
    ).astype(np.float32)
    if np.any(b != 0):
        full = full + b[None, :]
    return full
